# revision 1
# baseline (speedup 1.0000x reference)
"""Trainium2 Bass kernel for the three-GEU (text/video/audio) embedding model.

Strategy (8 NeuronCores, one chip):
  - Tensor-parallel column sharding: core c owns output columns [512c, 512(c+1))
    of every linear; it reads only its 1/8 slice of each weight matrix
    (weights dominate DMA: ~22 MB fp16 per core = the per-core HBM roofline).
  - Preprocessing (text max-pool over L, audio ragged masked-mean over T) is
    sharded over the feature dim, computed in transposed layout, and an
    AllGather assembles the full [K, B] activations every core needs as the
    matmul stationary operand.
  - Each GEU: GEMM1 -> PE-transpose h -> AllGather(hT) -> GEMM2 -> sigmoid,
    y = h * sig(g), partial sum(y^2); one tiny AllReduce of the three norm
    partials, then rsqrt scaling on-device.
  - fp16 operands into the PE (fp32 PSUM accumulation), fp32 outputs.
"""

import numpy as np

B = 64
L = 30
D = 4096
DA = 1024
T = 128
NCORES = 8
S = D // NCORES     # 512: per-core output shard of D
SA = DA // NCORES   # 128: per-core shard of Da
KD = D // 128       # 32 k-tiles over D
KA = DA // 128      # 8 k-tiles over Da
CH = 16             # k-tiles per weight DMA chunk (16 * 128 * 512 * 2B = 2 MiB)

_STATE: dict = {}


def _build():
    from contextlib import ExitStack

    import concourse.bass as bass
    import concourse.tile as tile
    from concourse import bacc, mybir
    from concourse.bass import ts
    from concourse.masks import make_identity

    fp16 = mybir.dt.float16
    f32 = mybir.dt.float32
    AX = mybir.AxisListType
    ALU = mybir.AluOpType
    ACTF = mybir.ActivationFunctionType

    nc = bacc.Bacc(
        "TRN2",
        target_bir_lowering=False,
        debug=False,
        enable_asserts=False,
        num_devices=NCORES,
    )
    RG = [list(range(NCORES))]

    # --- kernel I/O (per-core shards, staged by the host wrapper) ---
    # Weights arrive pre-tiled [n_chunks, 128, CH, S] so each chunk DMA is
    # fully contiguous per partition (16 KB) — scattered 1 KB segments run
    # at ~57 GB/s vs ~350+ GB/s contiguous.
    w_in = {}
    for name, kk in [("wt", D), ("wgt", D), ("wv", D), ("wgv", D),
                     ("wga", D), ("wa", DA)]:
        nkt = kk // 128
        nch = max(1, nkt // CH)
        w_in[name] = nc.dram_tensor(
            name, [nch, 128, (nkt // nch) * S], fp16, kind="ExternalInput")
    textT = nc.dram_tensor("textT", [S, B, L], fp16, kind="ExternalInput")
    audioT = nc.dram_tensor("audioT", [T, B, SA], fp16, kind="ExternalInput")
    vT_d = nc.dram_tensor("vT", [128, KD, B], fp16, kind="ExternalInput")
    maskT_d = nc.dram_tensor("maskT", [T, B], fp16, kind="ExternalInput")
    biases_d = nc.dram_tensor("biases", [1, 6 * S], fp16, kind="ExternalInput")
    EMBEDS = ("text", "video", "audio")
    out_d = {
        e: nc.dram_tensor(f"out_{e}", [B, S], f32, kind="ExternalOutput")
        for e in EMBEDS
    }
    import os
    _DBG = bool(os.environ.get("KBASS_DEBUG"))
    if _DBG:
        dbg_nsq = nc.dram_tensor("dbg_nsq", [B, 4], f32, kind="ExternalOutput")
        dbg_nsqg = nc.dram_tensor("dbg_nsqg", [B, 3], f32, kind="ExternalOutput")
        dbg_rcp = nc.dram_tensor("dbg_rcp", [B, 3], f32, kind="ExternalOutput")
        dbg_h = {e: nc.dram_tensor(f"dbg_h_{e}", [B, S], f32,
                                   kind="ExternalOutput") for e in EMBEDS}
        dbg_sg = {e: nc.dram_tensor(f"dbg_sg_{e}", [B, S], f32,
                                    kind="ExternalOutput") for e in EMBEDS}
        dbg_acts = nc.dram_tensor("dbg_acts", [128, 40, B], f32,
                                  kind="ExternalOutput")
        dbg_stg = nc.dram_tensor("dbg_stg", [128, 5, B], f32,
                                 kind="ExternalOutput")
        dbg_vt = nc.dram_tensor("dbg_vt", [128, KD, B], f32,
                                kind="ExternalOutput")
        dbg_msk = nc.dram_tensor("dbg_msk", [T, B], f32,
                                 kind="ExternalOutput")

    BIAS_IDX = {("text", 1): 0, ("text", 2): 1, ("video", 1): 2,
                ("video", 2): 3, ("audio", 1): 4, ("audio", 2): 5}

    with ExitStack() as ctx:
        tc = ctx.enter_context(tile.TileContext(nc))

        persist = ctx.enter_context(tc.tile_pool(name="persist", bufs=1))
        wpool = ctx.enter_context(tc.tile_pool(name="wstream", bufs=6))
        work = ctx.enter_context(tc.tile_pool(name="work", bufs=2))
        psum = ctx.enter_context(tc.tile_pool(name="psum", bufs=2, space="PSUM"))
        dram = ctx.enter_context(tc.tile_pool(name="dram", bufs=1, space="DRAM"))

        # ---- persistent SBUF tiles ----
        acts_all = persist.tile([128, 40, B], fp16)   # AG1 result: 8 chunks x (4 text + 1 audio) k-tiles
        au_sb = persist.tile([T, B, SA], fp16)        # audio shard, [t, b, c']
        vt_sb = persist.tile([128, KD, B], fp16)      # video.T k-tiles
        msk_sb = persist.tile([T, B], fp16)           # mask/nf, transposed
        bias_sb = persist.tile([1, 6, S], fp16)
        ones_sb = persist.tile([1, B], fp16)
        ident = persist.tile([B, B], fp16)
        stg = persist.tile([128, 5, B], fp16)         # AG1 staging: 4 text tiles + 1 audio tile
        nsq = persist.tile([B, 4], f32)               # partial sum(y^2) per embed
        nsqg = persist.tile([B, 3], f32)              # AllReduce result
        nrm = persist.tile([B, 3], f32)
        rcp = persist.tile([B, 3], f32)
        # combined transposed-h staging / gathered result (one AllGather)
        hstg = persist.tile([128, 3, 4, B], fp16)
        hT_all = persist.tile([128, NCORES, 3, 4, B], fp16)
        h16 = {e: persist.tile([B, S], fp16, name=f"h16_{e}") for e in EMBEDS}
        y_sb = {e: persist.tile([B, S], f32, name=f"y_{e}") for e in EMBEDS}

        # ---- constants ----
        nc.gpsimd.memset(ones_sb[:], 1.0)
        nc.vector.memset(nsq[:], 0.0)
        make_identity(nc, ident[:])

        # ---- input DMAs: audio/mask on sync, video/bias on scalar (start at
        # t=0 ahead of the weight chunks), text on the SWDGE ring ----
        nc.sync.dma_start(au_sb[:], audioT.ap())
        nc.sync.dma_start(msk_sb[:], maskT_d.ap())
        nc.scalar.dma_start(bias_sb[0:1, :, :], biases_d.ap())
        nc.scalar.dma_start(vt_sb[:], vT_d.ap())

        # ---- text max-pool over L (sharded over d) -> stg[:, 0:4, :] ----
        t_view = textT.ap().rearrange("(n p) b l -> n p b l", p=128)
        for i in range(4):
            tx = work.tile([128, B, L], fp16, name="tx", bufs=4)
            nc.gpsimd.dma_start(tx[:], t_view[i])
            nc.vector.reduce_max(stg[:, i, :], tx[:], AX.X)

        # ---- audio ragged masked-mean (sharded over Da): 64 PE matvecs ----
        aT_ps = psum.tile([SA, B], f32, bufs=1)
        for b in range(B):
            nc.tensor.matmul(
                aT_ps[:, b:b + 1], au_sb[:, b, :], msk_sb[:, b:b + 1],
                start=True, stop=True)
        nc.vector.tensor_copy(stg[:, 4, :], aT_ps[:])

        # ---- AllGather the preprocessed activations ----
        # Row p of ag1_in carries partition p's 5 staged k-tile rows flat
        # (5*B els), so staging and reload DMAs are contiguous per partition.
        ag1_in = dram.tile([128, 5 * B], fp16)
        ag1_out = dram.tile([128 * NCORES, 5 * B], fp16, addr_space="Shared")
        nc.gpsimd.dma_start(ag1_in[:], stg[:])
        nc.gpsimd.collective_compute(
            "AllGather", ALU.bypass, replica_groups=RG,
            ins=[ag1_in.opt()], outs=[ag1_out.opt()])
        nc.gpsimd.dma_start(
            acts_all.rearrange("p (r s) b -> p r (s b)", s=5),
            ag1_out.rearrange("(r p) sb -> p r sb", p=128))
        if _DBG:
            nc.gpsimd.dma_start(dbg_acts.ap(), acts_all[:])
            nc.gpsimd.dma_start(dbg_stg.ap(), stg[:])
            nc.gpsimd.dma_start(dbg_vt.ap(), vt_sb[:])
            nc.gpsimd.dma_start(dbg_msk.ap(), msk_sb[:])

        # lhsT accessors (stationary [128, B] k-tiles, transposed activations)
        def lhs_text(k):
            return acts_all[:, (k // 4) * 5 + (k % 4), :]

        def lhs_audio(k):
            return acts_all[:, k * 5 + 4, :]

        def lhs_video(k):
            return vt_sb[:, k, :]

        hwdge = [nc.sync, nc.scalar]
        chunk_no = [0]

        def gemm(out_ps, w_dram, n_kt, lhs_fn, bias_idx):
            # bias as a K=1 matmul row; also opens the accumulation group
            nc.tensor.matmul(out_ps[:], ones_sb[:], bias_sb[:, bias_idx, :],
                             start=True, stop=False)
            nch = w_dram.shape[0]
            cnt = n_kt // nch
            for ch in range(nch):
                w = wpool.tile([128, cnt, S], fp16, name="wchunk", tag="wchunk")
                eng = hwdge[chunk_no[0] % 2]
                chunk_no[0] += 1
                eng.dma_start(
                    w[:], w_dram.ap()[ch].rearrange("p (a n) -> p a n", n=S))
                for a in range(cnt):
                    k = ch * cnt + a
                    nc.tensor.matmul(out_ps[:], lhs_fn(k), w[:, a, :],
                                     start=False, stop=(k == n_kt - 1))

        _STAGE = int(os.environ.get("KBASS_STAGE", "3"))

        # ---- stage 1: the three first linears + one combined hT AllGather ----
        W1 = {"text": ("wt", KD, lhs_text), "video": ("wv", KD, lhs_video),
              "audio": ("wa", KA, lhs_audio)}
        for ei, e in enumerate(EMBEDS):
            wname, nkt, lf = W1[e]
            h_ps = psum.tile([B, S], f32, name="h_ps", tag="h_ps")
            gemm(h_ps, w_in[wname], nkt, lf, BIAS_IDX[(e, 1)])
            nc.vector.tensor_copy(h16[e][:], h_ps[:])
            if _STAGE < 2:
                continue
            # transpose h shard ([B, S] -> 4 x [128, B]) on the PE
            hT_ps = psum.tile([128, 4, B], fp16, name="hT_ps", tag="hT_ps",
                              bufs=1)
            for j in range(4):
                nc.tensor.transpose(hT_ps[:, j, :], h16[e][:, ts(j, 128)],
                                    ident[:])
            nc.vector.tensor_copy(hstg[:, ei, :, :], hT_ps[:])

        if _STAGE >= 3:
            agh_in = dram.tile([128, 3 * 4 * B], fp16)
            agh_out = dram.tile([128 * NCORES, 3 * 4 * B], fp16,
                                addr_space="Shared")
            nc.gpsimd.dma_start(agh_in[:], hstg[:])
            nc.gpsimd.collective_compute(
                "AllGather", ALU.bypass, replica_groups=RG,
                ins=[agh_in.opt()], outs=[agh_out.opt()])
            nc.gpsimd.dma_start(
                hT_all.rearrange("p r e j b -> p r (e j b)"),
                agh_out.rearrange("(r p) x -> p r x", p=128))

            # ---- stage 2: gating linears, GLU, partial norms ----
            W2 = {"text": "wgt", "video": "wgv", "audio": "wga"}
            for ei, e in enumerate(EMBEDS):
                g_ps = psum.tile([B, S], f32, name="g_ps", tag="g_ps")
                gemm(g_ps, w_in[W2[e]], KD,
                     lambda k, ei=ei: hT_all[:, k // 4, ei, k % 4, :],
                     BIAS_IDX[(e, 2)])
                sg16 = work.tile([B, S], fp16, name="sg16", tag="sg16")
                nc.scalar.activation(sg16[:], g_ps[:], ACTF.Sigmoid)
                nc.vector.tensor_mul(y_sb[e][:], h16[e][:], sg16[:])
                if _DBG:
                    nc.gpsimd.dma_start(dbg_h[e].ap(), h16[e][:])
                    nc.gpsimd.dma_start(dbg_sg[e].ap(), sg16[:])
                ysq = work.tile([B, S], f32, name="ysq", tag="ysq")
                nc.vector.tensor_mul(ysq[:], y_sb[e][:], y_sb[e][:])
                nc.vector.reduce_sum(nsq[:, ei:ei + 1], ysq[:], AX.X)

            # ---- AllReduce norm partials; normalize; write outputs ----
            ar_in = dram.tile([B, 3], f32)
            ar_out = dram.tile([B, 3], f32, addr_space="Shared")
            nc.gpsimd.dma_start(ar_in[:], nsq[:, 0:3])
            nc.gpsimd.collective_compute(
                "AllReduce", ALU.add, replica_groups=RG,
                ins=[ar_in.opt()], outs=[ar_out.opt()])
            nc.gpsimd.dma_start(nsqg[:], ar_out[:])
            nc.scalar.sqrt(nrm[:], nsqg[:])
            nc.vector.tensor_scalar_max(nrm[:], nrm[:], 1e-12)
            nc.vector.reciprocal(rcp[:], nrm[:])
            if _DBG:
                nc.sync.dma_start(dbg_nsq.ap(), nsq[:])
                nc.sync.dma_start(dbg_nsqg.ap(), nsqg[:])
                nc.sync.dma_start(dbg_rcp.ap(), rcp[:])
            for ei, e in enumerate(EMBEDS):
                yo = work.tile([B, S], f32, name="yo", tag="yo")
                nc.vector.tensor_scalar_mul(yo[:], y_sb[e][:],
                                            rcp[:, ei:ei + 1])
                nc.sync.dma_start(out_d[e].ap(), yo[:])
        else:
            for e in EMBEDS:
                nc.gpsimd.dma_start(out_d[e].ap(), h16[e][:])

    nc.compile()
    return nc


def _get_nc():
    if "nc" not in _STATE:
        _STATE["nc"] = _build()
    return _STATE["nc"]


def _prep_inputs(text, video, audio_feats, Wt, bt, Wgt, bgt, Wv, bv, Wgv, bgv,
                 Wa, ba, Wga, bga, nframes, raw_audio_len):
    """Shard + transpose + fp16-cast the full inputs into per-core in_maps."""
    f16 = np.float16
    text = np.asarray(text, dtype=np.float32)
    video = np.asarray(video, dtype=np.float32)
    audio = np.asarray(audio_feats, dtype=np.float32)

    ratio = int(round(float(np.asarray(raw_audio_len)) / T))
    nf = np.maximum(
        1, (np.asarray(nframes).astype(np.float32) / ratio).astype(np.int32))
    mask = (np.arange(T)[None, :] < nf[:, None]).astype(np.float32)
    mask = mask / nf[:, None].astype(np.float32)          # [B, T] mask/nf
    maskT = np.ascontiguousarray(mask.T).astype(f16)      # [T, B]

    # video.T pre-tiled to [128, KD, B] (partition-contiguous k-tiles)
    vT = np.ascontiguousarray(
        video.T.reshape(KD, 128, B).transpose(1, 0, 2)).astype(f16)

    def wtile(W, sl):
        """W[sl].T [K, S] -> chunked [nch, 128, cnt*S], contiguous/partition."""
        wt = W[sl, :].T
        kk = wt.shape[0]
        nkt = kk // 128
        nch = max(1, nkt // CH)
        cnt = nkt // nch
        return np.ascontiguousarray(
            wt.reshape(nch, cnt, 128, S).transpose(0, 2, 1, 3)
            .reshape(nch, 128, cnt * S)).astype(f16)

    in_maps = []
    for c in range(NCORES):
        sl = slice(c * S, (c + 1) * S)
        sla = slice(c * SA, (c + 1) * SA)
        m = {
            "wt": wtile(Wt, sl),
            "wgt": wtile(Wgt, sl),
            "wv": wtile(Wv, sl),
            "wgv": wtile(Wgv, sl),
            "wga": wtile(Wga, sl),
            "wa": wtile(Wa, sl),
            "textT": np.ascontiguousarray(
                text[:, :, sl].transpose(2, 0, 1)).astype(f16),
            "audioT": np.ascontiguousarray(
                audio[:, sla, :].transpose(2, 0, 1)).astype(f16),
            "vT": vT,
            "maskT": maskT,
            "biases": np.stack([
                np.asarray(b, dtype=np.float32)[sl] for b in
                (bt, bgt, bv, bgv, ba, bga)
            ]).reshape(1, -1).astype(f16),
        }
        in_maps.append(m)
    return in_maps


def kernel(text, video, audio_feats, Wt, bt, Wgt, bgt, Wv, bv, Wgv, bgv,
           Wa, ba, Wga, bga, nframes, raw_audio_len):
    from concourse.bass_utils import run_bass_kernel_spmd

    nc = _get_nc()
    in_maps = _prep_inputs(text, video, audio_feats, Wt, bt, Wgt, bgt,
                           Wv, bv, Wgv, bgv, Wa, ba, Wga, bga,
                           nframes, raw_audio_len)
    res = run_bass_kernel_spmd(nc, in_maps, list(range(NCORES)))
    _STATE["last_results"] = res
    outs = []
    for e in ("text", "video", "audio"):
        outs.append(np.concatenate(
            [res.results[c][f"out_{e}"] for c in range(NCORES)], axis=1))
    return tuple(outs)



# revision 2
# speedup vs baseline: 1.2510x; 1.2510x over previous
"""Trainium2 Bass kernel for the three-GEU (text/video/audio) embedding model.

Strategy (8 NeuronCores, one chip):
  - Tensor-parallel column sharding: core c owns output columns [512c, 512(c+1))
    of every linear; it reads only its 1/8 slice of each weight matrix.
  - Gating fusion (host): g = (x@W.T+b)@Wg.T+bg == x@(Wg@W).T + (bg+Wg@b),
    so the gating GEMM reads the same gathered activations as the value GEMM.
    This removes the mid-kernel h-transpose + AllGather of the baseline and
    shrinks the audio gating weight from [4096,4096] to [4096,1024].
  - Preprocessing (text max-pool over L, audio ragged masked-mean over T) is
    sharded over the feature dim, computed in transposed layout, and an
    AllGather assembles the full [K, B] activations every core needs as the
    matmul stationary operand. This is the FIRST collective: it absorbs the
    inter-core launch skew while the weight stream saturates DMA.
  - Per embed: h = GEMM(x, Wslice), g = GEMM(x, Wfused_slice), y = h*sig(g),
    partial sum(y^2); one tiny AllReduce of the three norm partials, then
    reciprocal-sqrt scaling on-device.
  - fp16 operands into the PE (fp32 PSUM accumulation), fp32 outputs.
  - Video's GEMMs depend only on local inputs, so they run before the
    AllGather completes; PE order: audio matvecs, video h+g, audio h+g,
    text h+g (audio weights stream before text's so the PE is never starved).
"""

import numpy as np

B = 64
L = 30
D = 4096
DA = 1024
T = 128
NCORES = 8
S = D // NCORES     # 512: per-core output shard of D
SA = DA // NCORES   # 128: per-core shard of Da
KD = D // 128       # 32 k-tiles over D
KA = DA // 128      # 8 k-tiles over Da
CH = 16             # k-tiles per weight DMA chunk (16 * 128 * 512 * 2B = 2 MiB)

_STATE: dict = {}


def _build():
    from contextlib import ExitStack

    import concourse.bass as bass
    import concourse.tile as tile
    from concourse import bacc, mybir

    fp16 = mybir.dt.float16
    f32 = mybir.dt.float32
    AX = mybir.AxisListType
    ALU = mybir.AluOpType
    ACTF = mybir.ActivationFunctionType

    nc = bacc.Bacc(
        "TRN2",
        target_bir_lowering=False,
        debug=False,
        enable_asserts=False,
        num_devices=NCORES,
    )
    RG = [list(range(NCORES))]

    # --- kernel I/O (per-core shards, staged by the host wrapper) ---
    # Weights arrive pre-tiled [n_chunks, 128, CH, S] so each chunk DMA is
    # fully contiguous per partition (16 KB).
    w_in = {}
    for name, kk in [("wv", D), ("wgv", D), ("wa", DA), ("wga", DA),
                     ("wt", D), ("wgt", D)]:
        nkt = kk // 128
        nch = max(1, nkt // CH)
        w_in[name] = nc.dram_tensor(
            name, [nch, 128, (nkt // nch) * S], fp16, kind="ExternalInput")
    textT = nc.dram_tensor("textT", [S, B, L], fp16, kind="ExternalInput")
    audioT = nc.dram_tensor("audioT", [T, B, SA], fp16, kind="ExternalInput")
    vT_d = nc.dram_tensor("vT", [128, KD, B], fp16, kind="ExternalInput")
    maskT_d = nc.dram_tensor("maskT", [T, B], fp16, kind="ExternalInput")
    biases_d = nc.dram_tensor("biases", [1, 6 * S], fp16, kind="ExternalInput")
    EMBEDS = ("text", "video", "audio")
    out_d = {
        e: nc.dram_tensor(f"out_{e}", [B, S], f32, kind="ExternalOutput")
        for e in EMBEDS
    }

    BIAS_IDX = {("text", 1): 0, ("text", 2): 1, ("video", 1): 2,
                ("video", 2): 3, ("audio", 1): 4, ("audio", 2): 5}

    with ExitStack() as ctx:
        tc = ctx.enter_context(tile.TileContext(nc))

        persist = ctx.enter_context(tc.tile_pool(name="persist", bufs=1))
        work = ctx.enter_context(tc.tile_pool(name="work", bufs=2))
        psum = ctx.enter_context(tc.tile_pool(name="psum", bufs=2, space="PSUM"))
        dram = ctx.enter_context(tc.tile_pool(name="dram", bufs=1, space="DRAM"))

        # ---- persistent SBUF tiles ----
        # All weights live in SBUF for the whole kernel (18 MB of the 26 MB
        # budget) so the PE never waits on a buffer-rotation dependency.
        w_sb = {}
        for name in ("wv", "wgv", "wa", "wga", "wt", "wgt"):
            nch, _, csz = w_in[name].shape
            w_sb[name] = persist.tile([128, nch, csz], fp16, name=f"w_{name}")
        acts_all = persist.tile([128, 40, B], fp16)   # AG result: 8 cores x (4 text + 1 audio) k-tiles
        au_sb = persist.tile([T, B, SA], fp16)        # audio shard, [t, b, c']
        vt_sb = persist.tile([128, KD, B], fp16)      # video.T k-tiles
        msk_sb = persist.tile([T, B], fp16)           # mask/nf, transposed
        bias_sb = persist.tile([1, 6, S], fp16)
        ones_sb = persist.tile([1, B], fp16)
        stg = persist.tile([128, 5, B], fp16)         # AG staging: 4 text + 1 audio k-tile rows
        nsq = persist.tile([B, 4], f32)               # partial sum(y^2) per embed
        nsqg = persist.tile([B, 3], f32)              # AllReduce result
        nrm = persist.tile([B, 3], f32)
        rcp = persist.tile([B, 3], f32)
        h16 = {e: persist.tile([B, S], fp16, name=f"h16_{e}") for e in EMBEDS}
        y_sb = {e: persist.tile([B, S], f32, name=f"y_{e}") for e in EMBEDS}

        # ---- constants ----
        nc.gpsimd.memset(ones_sb[:], 1.0)
        nc.vector.memset(nsq[:], 0.0)

        # ---- input DMAs: audio/mask first on sync, video/bias first on
        # scalar (both HWDGE queues), text on the SWDGE ring ----
        nc.sync.dma_start(au_sb[:], audioT.ap())
        nc.sync.dma_start(msk_sb[:], maskT_d.ap())
        nc.scalar.dma_start(bias_sb[0:1, :, :], biases_d.ap())
        nc.scalar.dma_start(vt_sb[:], vT_d.ap())

        # ---- weight stream: all chunks up-front, in PE consumption order,
        # alternating between the two HWDGE queues ----
        hwdge = [nc.sync, nc.scalar]
        qi = 0
        for name in ("wv", "wgv", "wa", "wga", "wt", "wgt"):
            nch = w_in[name].shape[0]
            for ch in range(nch):
                hwdge[qi % 2].dma_start(
                    w_sb[name][:, ch, :], w_in[name].ap()[ch])
                qi += 1

        # ---- text max-pool over L (sharded over d) -> stg[:, 0:4, :] ----
        t_view = textT.ap().rearrange("(n p) b l -> n p b l", p=128)
        for i in range(4):
            tx = work.tile([128, B, L], fp16, name="tx", bufs=2)
            nc.gpsimd.dma_start(tx[:], t_view[i])
            nc.vector.reduce_max(stg[:, i, :], tx[:], AX.X)

        # ---- audio ragged masked-mean (sharded over Da): 64 PE matvecs ----
        aT_ps = psum.tile([SA, B], f32, bufs=1)
        for b in range(B):
            nc.tensor.matmul(
                aT_ps[:, b:b + 1], au_sb[:, b, :], msk_sb[:, b:b + 1],
                start=True, stop=True)
        nc.vector.tensor_copy(stg[:, 4, :], aT_ps[:])

        # ---- AllGather the preprocessed activations ----
        # Row p of ag1_in carries partition p's 5 staged k-tile rows flat
        # (5*B els), so staging and reload DMAs are contiguous per partition.
        ag1_in = dram.tile([128, 5 * B], fp16)
        ag1_out = dram.tile([128 * NCORES, 5 * B], fp16, addr_space="Shared")
        nc.gpsimd.dma_start(ag1_in[:], stg[:])
        nc.gpsimd.collective_compute(
            "AllGather", ALU.bypass, replica_groups=RG,
            ins=[ag1_in.opt()], outs=[ag1_out.opt()])
        # reload audio rows first (audio GEMMs run before text's)
        ag1_v = ag1_out.rearrange("(r p) (j b) -> p r j b", p=128, b=B)
        nc.gpsimd.dma_start(
            acts_all.rearrange("p (r j) b -> p r j b", j=5)[:, :, 4, :],
            ag1_v[:, :, 4, :])
        nc.gpsimd.dma_start(
            acts_all.rearrange("p (r j) b -> p r j b", j=5)[:, :, 0:4, :],
            ag1_v[:, :, 0:4, :])

        # lhsT accessors (stationary [128, B] k-tiles, transposed activations)
        def lhs_text(k):
            return acts_all[:, (k // 4) * 5 + (k % 4), :]

        def lhs_audio(k):
            return acts_all[:, k * 5 + 4, :]

        def lhs_video(k):
            return vt_sb[:, k, :]

        def gemm(out_ps, wname, n_kt, lhs_fn, bias_idx):
            # bias as a K=1 matmul row; also opens the accumulation group
            nc.tensor.matmul(out_ps[:], ones_sb[:], bias_sb[:, bias_idx, :],
                             start=True, stop=False)
            w = w_sb[wname]
            nch = w.shape[1]
            cnt = n_kt // nch
            wv = w.rearrange("p c (a n) -> p c a n", n=S)
            for ch in range(nch):
                for a in range(cnt):
                    k = ch * cnt + a
                    nc.tensor.matmul(out_ps[:], lhs_fn(k), wv[:, ch, a, :],
                                     start=False, stop=(k == n_kt - 1))

        # ---- the six GEMMs + GLU/partial-norm per embed ----
        # (video first: its inputs are local, so it runs during the skew
        # window while AG1 is still in flight)
        PLAN = [("video", "wv", "wgv", KD, lhs_video),
                ("audio", "wa", "wga", KA, lhs_audio),
                ("text", "wt", "wgt", KD, lhs_text)]
        EIDX = {e: i for i, e in enumerate(EMBEDS)}
        for e, wn1, wn2, nkt, lf in PLAN:
            ei = EIDX[e]
            h_ps = psum.tile([B, S], f32, name="h_ps", tag="h_ps")
            gemm(h_ps, wn1, nkt, lf, BIAS_IDX[(e, 1)])
            nc.vector.tensor_copy(h16[e][:], h_ps[:])
            g_ps = psum.tile([B, S], f32, name="g_ps", tag="g_ps")
            gemm(g_ps, wn2, nkt, lf, BIAS_IDX[(e, 2)])
            sg16 = work.tile([B, S], fp16, name="sg16", tag="sg16")
            nc.scalar.activation(sg16[:], g_ps[:], ACTF.Sigmoid)
            nc.vector.tensor_mul(y_sb[e][:], h16[e][:], sg16[:])
            ysq = work.tile([B, S], f32, name="ysq", tag="ysq")
            nc.vector.tensor_mul(ysq[:], y_sb[e][:], y_sb[e][:])
            nc.vector.reduce_sum(nsq[:, ei:ei + 1], ysq[:], AX.X)

        # ---- AllReduce norm partials; normalize; write outputs ----
        ar_in = dram.tile([B, 3], f32)
        ar_out = dram.tile([B, 3], f32, addr_space="Shared")
        nc.gpsimd.dma_start(ar_in[:], nsq[:, 0:3])
        nc.gpsimd.collective_compute(
            "AllReduce", ALU.add, replica_groups=RG,
            ins=[ar_in.opt()], outs=[ar_out.opt()])
        nc.gpsimd.dma_start(nsqg[:], ar_out[:])
        nc.scalar.sqrt(nrm[:], nsqg[:])
        nc.vector.tensor_scalar_max(nrm[:], nrm[:], 1e-12)
        nc.vector.reciprocal(rcp[:], nrm[:])
        oq = [nc.sync, nc.scalar, nc.gpsimd]
        for ei, e in enumerate(EMBEDS):
            yo = work.tile([B, S], f32, name="yo", tag="yo")
            nc.vector.tensor_scalar_mul(yo[:], y_sb[e][:],
                                        rcp[:, ei:ei + 1])
            oq[ei].dma_start(out_d[e].ap(), yo[:])

    nc.compile()
    return nc


def _get_nc():
    if "nc" not in _STATE:
        _STATE["nc"] = _build()
    return _STATE["nc"]


def _prep_inputs(text, video, audio_feats, Wt, bt, Wgt, bgt, Wv, bv, Wgv, bgv,
                 Wa, ba, Wga, bga, nframes, raw_audio_len):
    """Fuse gating weights, shard + transpose + fp16-cast into per-core maps."""
    f16 = np.float16
    text = np.asarray(text, dtype=np.float32)
    video = np.asarray(video, dtype=np.float32)
    audio = np.asarray(audio_feats, dtype=np.float32)

    Wt = np.asarray(Wt, dtype=np.float32)
    Wgt = np.asarray(Wgt, dtype=np.float32)
    Wv = np.asarray(Wv, dtype=np.float32)
    Wgv = np.asarray(Wgv, dtype=np.float32)
    Wa = np.asarray(Wa, dtype=np.float32)
    Wga = np.asarray(Wga, dtype=np.float32)
    bt = np.asarray(bt, dtype=np.float32)
    bgt = np.asarray(bgt, dtype=np.float32)
    bv = np.asarray(bv, dtype=np.float32)
    bgv = np.asarray(bgv, dtype=np.float32)
    ba = np.asarray(ba, dtype=np.float32)
    bga = np.asarray(bga, dtype=np.float32)

    # gating fusion: g = x @ (Wg@W).T + (bg + Wg@b)
    Wgt_f = Wgt @ Wt
    bgt_f = bgt + Wgt @ bt
    Wgv_f = Wgv @ Wv
    bgv_f = bgv + Wgv @ bv
    Wga_f = Wga @ Wa                     # [D, Da]
    bga_f = bga + Wga @ ba

    ratio = int(round(float(np.asarray(raw_audio_len)) / T))
    nf = np.maximum(
        1, (np.asarray(nframes).astype(np.float32) / ratio).astype(np.int32))
    mask = (np.arange(T)[None, :] < nf[:, None]).astype(np.float32)
    mask = mask / nf[:, None].astype(np.float32)          # [B, T] mask/nf
    maskT = np.ascontiguousarray(mask.T).astype(f16)      # [T, B]

    # video.T pre-tiled to [128, KD, B] (partition-contiguous k-tiles)
    vT = np.ascontiguousarray(
        video.T.reshape(KD, 128, B).transpose(1, 0, 2)).astype(f16)

    def wtile(W, sl):
        """W[sl].T [K, S] -> chunked [nch, 128, cnt*S], contiguous/partition."""
        wt = W[sl, :].T
        kk = wt.shape[0]
        nkt = kk // 128
        nch = max(1, nkt // CH)
        cnt = nkt // nch
        return np.ascontiguousarray(
            wt.reshape(nch, cnt, 128, S).transpose(0, 2, 1, 3)
            .reshape(nch, 128, cnt * S)).astype(f16)

    in_maps = []
    for c in range(NCORES):
        sl = slice(c * S, (c + 1) * S)
        sla = slice(c * SA, (c + 1) * SA)
        m = {
            "wt": wtile(Wt, sl),
            "wgt": wtile(Wgt_f, sl),
            "wv": wtile(Wv, sl),
            "wgv": wtile(Wgv_f, sl),
            "wga": wtile(Wga_f, sl),
            "wa": wtile(Wa, sl),
            "textT": np.ascontiguousarray(
                text[:, :, sl].transpose(2, 0, 1)).astype(f16),
            "audioT": np.ascontiguousarray(
                audio[:, sla, :].transpose(2, 0, 1)).astype(f16),
            "vT": vT,
            "maskT": maskT,
            "biases": np.stack([
                b[sl] for b in (bt, bgt_f, bv, bgv_f, ba, bga_f)
            ]).reshape(1, -1).astype(f16),
        }
        in_maps.append(m)
    return in_maps


def kernel(text, video, audio_feats, Wt, bt, Wgt, bgt, Wv, bv, Wgv, bgv,
           Wa, ba, Wga, bga, nframes, raw_audio_len):
    from concourse.bass_utils import run_bass_kernel_spmd

    nc = _get_nc()
    in_maps = _prep_inputs(text, video, audio_feats, Wt, bt, Wgt, bgt,
                           Wv, bv, Wgv, bgv, Wa, ba, Wga, bga,
                           nframes, raw_audio_len)
    res = run_bass_kernel_spmd(nc, in_maps, list(range(NCORES)))
    _STATE["last_results"] = res
    outs = []
    for e in ("text", "video", "audio"):
        outs.append(np.concatenate(
            [res.results[c][f"out_{e}"] for c in range(NCORES)], axis=1))
    return tuple(outs)


# revision 6
# speedup vs baseline: 1.2608x; 1.0078x over previous
"""Trainium2 Bass kernel for the three-GEU (text/video/audio) embedding model.

Strategy (8 NeuronCores, one chip):
  - Tensor-parallel column sharding: core c owns output columns [512c, 512(c+1))
    of every linear; it reads only its 1/8 slice of each weight matrix.
  - Gating fusion (host): g = (x@W.T+b)@Wg.T+bg == x@(Wg@W).T + (bg+Wg@b),
    so the gating GEMM reads the same gathered activations as the value GEMM.
    This removes the mid-kernel h-transpose + AllGather of the baseline and
    shrinks the audio gating weight from [4096,4096] to [4096,1024].
  - Preprocessing (text max-pool over L, audio ragged masked-mean over T) is
    sharded over the feature dim, computed in transposed layout, and an
    AllGather assembles the full [K, B] activations every core needs as the
    matmul stationary operand. This is the FIRST collective: it absorbs the
    inter-core launch skew while the weight stream saturates DMA.
  - Per embed: h = GEMM(x, Wslice), g = GEMM(x, Wfused_slice), y = h*sig(g),
    partial sum(y^2); one tiny AllReduce of the three norm partials, then
    reciprocal-sqrt scaling on-device.
  - fp16 operands into the PE (fp32 PSUM accumulation), fp32 outputs.
  - Video's GEMMs depend only on local inputs, so they run before the
    AllGather completes; PE order: audio matvecs, video h+g, audio h+g,
    text h+g (audio weights stream before text's so the PE is never starved).
"""

import numpy as np

B = 64
L = 30
D = 4096
DA = 1024
T = 128
NCORES = 8
S = D // NCORES     # 512: per-core output shard of D
SA = DA // NCORES   # 128: per-core shard of Da
KD = D // 128       # 32 k-tiles over D
KA = DA // 128      # 8 k-tiles over Da
CH = 16             # k-tiles per weight DMA chunk (16 * 128 * 512 * 2B = 2 MiB)

_STATE: dict = {}


def _build():
    from contextlib import ExitStack

    import concourse.bass as bass
    import concourse.tile as tile
    from concourse import bacc, mybir

    fp16 = mybir.dt.float16
    f32 = mybir.dt.float32
    AX = mybir.AxisListType
    ALU = mybir.AluOpType
    ACTF = mybir.ActivationFunctionType

    nc = bacc.Bacc(
        "TRN2",
        target_bir_lowering=False,
        debug=False,
        enable_asserts=False,
        num_devices=NCORES,
    )
    RG = [list(range(NCORES))]

    # --- kernel I/O (per-core shards, staged by the host wrapper) ---
    # Weights arrive pre-tiled [n_chunks, 128, CH, S] so each chunk DMA is
    # fully contiguous per partition (16 KB).
    w_in = {}
    for name, kk in [("wv", D), ("wgv", D), ("wa", DA), ("wga", DA),
                     ("wt", D), ("wgt", D)]:
        nkt = kk // 128
        nch = max(1, nkt // CH)
        w_in[name] = nc.dram_tensor(
            name, [nch, 128, (nkt // nch) * S], fp16, kind="ExternalInput")
    textT = nc.dram_tensor("textT", [S, B, L], fp16, kind="ExternalInput")
    audioT = nc.dram_tensor("audioT", [T, B, SA], fp16, kind="ExternalInput")
    vT_d = nc.dram_tensor("vT", [128, KD, B], fp16, kind="ExternalInput")
    maskT_d = nc.dram_tensor("maskT", [T, B], fp16, kind="ExternalInput")
    biases_d = nc.dram_tensor("biases", [1, 6 * S], fp16, kind="ExternalInput")
    EMBEDS = ("text", "video", "audio")
    out_d = {
        e: nc.dram_tensor(f"out_{e}", [B, S], f32, kind="ExternalOutput")
        for e in EMBEDS
    }

    BIAS_IDX = {("text", 1): 0, ("text", 2): 1, ("video", 1): 2,
                ("video", 2): 3, ("audio", 1): 4, ("audio", 2): 5}

    with ExitStack() as ctx:
        tc = ctx.enter_context(tile.TileContext(nc))

        persist = ctx.enter_context(tc.tile_pool(name="persist", bufs=1))
        work = ctx.enter_context(tc.tile_pool(name="work", bufs=2))
        psum = ctx.enter_context(tc.tile_pool(name="psum", bufs=2, space="PSUM"))
        dram = ctx.enter_context(tc.tile_pool(name="dram", bufs=1, space="DRAM"))

        # ---- persistent SBUF tiles ----
        # All weights live in SBUF for the whole kernel (18 MB of the 26 MB
        # budget) so the PE never waits on a buffer-rotation dependency.
        w_sb = {}
        for name in ("wv", "wgv", "wa", "wga", "wt", "wgt"):
            nch, _, csz = w_in[name].shape
            w_sb[name] = persist.tile([128, nch, csz], fp16, name=f"w_{name}")
        acts_all = persist.tile([128, 40, B], fp16)   # AG result: 8 cores x (4 text + 1 audio) k-tiles
        au_sb = persist.tile([T, B, SA], fp16)        # audio shard, [t, b, c']
        vt_sb = persist.tile([128, KD, B], fp16)      # video.T k-tiles
        msk_sb = persist.tile([T, B], fp16)           # mask/nf, transposed
        bias_sb = persist.tile([1, 6, S], fp16)
        ones_sb = persist.tile([1, B], fp16)
        stg = persist.tile([128, 5, B], fp16)         # AG staging: 4 text + 1 audio k-tile rows
        nsq = persist.tile([B, 4], f32)               # partial sum(y^2) per embed
        nsqg = persist.tile([B, 3], f32)              # AllReduce result
        nrm = persist.tile([B, 3], f32)
        rcp = persist.tile([B, 3], f32)
        h16 = {e: persist.tile([B, S], fp16, name=f"h16_{e}") for e in EMBEDS}
        y_sb = {e: persist.tile([B, S], f32, name=f"y_{e}") for e in EMBEDS}

        # ---- constants ----
        nc.vector.memset(ones_sb[:], 1.0)
        nc.vector.memset(nsq[:], 0.0)

        # ---- input DMAs: audio/mask first on sync, video/bias first on
        # scalar (both HWDGE queues), text on the SWDGE ring ----
        nc.sync.dma_start(au_sb[:], audioT.ap())
        nc.sync.dma_start(msk_sb[:], maskT_d.ap())
        nc.scalar.dma_start(bias_sb[0:1, :, :], biases_d.ap())
        nc.scalar.dma_start(vt_sb[:], vT_d.ap())

        # ---- weight stream: all chunks up-front, in PE consumption order,
        # alternating between the two HWDGE queues. Each queue first runs a
        # tiny scratch DMA that reads stg, so the weight burst starts only
        # after the preprocessing inputs have full DMA bandwidth — this both
        # speeds the AllGather trigger path and leaves the DMA engines quiet
        # during the collective itself. ----
        scr = dram.tile([1, 8], fp16)
        nc.sync.dma_start(scr[0:1, 0:4], stg[0:1, 0:1, 0:4])
        nc.scalar.dma_start(scr[0:1, 4:8], stg[0:1, 0:1, 4:8])
        hwdge = [nc.sync, nc.scalar]
        qi = 0
        for name in ("wv", "wgv", "wt", "wgt", "wa", "wga"):
            nch = w_in[name].shape[0]
            for ch in range(nch):
                hwdge[qi % 2].dma_start(
                    w_sb[name][:, ch, :], w_in[name].ap()[ch])
                qi += 1

        # ---- text max-pool over L (sharded over d) -> stg[:, 0:4, :] ----
        t_view = textT.ap().rearrange("(n p) b l -> n p b l", p=128)
        for i in range(4):
            tx = work.tile([128, B, L], fp16, name="tx", bufs=2)
            nc.gpsimd.dma_start(tx[:], t_view[i])
            nc.vector.reduce_max(stg[:, i, :], tx[:], AX.X)

        # ---- audio ragged masked-mean (sharded over Da): 64 PE matvecs ----
        aT_ps = psum.tile([SA, B], f32, bufs=1)
        for b in range(B):
            nc.tensor.matmul(
                aT_ps[:, b:b + 1], au_sb[:, b, :], msk_sb[:, b:b + 1],
                start=True, stop=True)
        nc.vector.tensor_copy(stg[:, 4, :], aT_ps[:])

        # ---- AllGather the preprocessed activations ----
        # Row p of ag1_in carries partition p's 5 staged k-tile rows flat
        # (5*B els), so staging and reload DMAs are contiguous per partition.
        ag1_in = dram.tile([128, 5 * B], fp16)
        ag1_out = dram.tile([128 * NCORES, 5 * B], fp16, addr_space="Shared")
        nc.gpsimd.dma_start(ag1_in[:], stg[:])
        nc.gpsimd.collective_compute(
            "AllGather", ALU.bypass, replica_groups=RG,
            ins=[ag1_in.opt()], outs=[ag1_out.opt()])
        # reload text rows first (text GEMMs run right after the gather;
        # audio's come last)
        ag1_v = ag1_out.rearrange("(r p) (j b) -> p r j b", p=128, b=B)
        nc.gpsimd.dma_start(
            acts_all.rearrange("p (r j) b -> p r j b", j=5)[:, :, 0:4, :],
            ag1_v[:, :, 0:4, :])
        nc.gpsimd.dma_start(
            acts_all.rearrange("p (r j) b -> p r j b", j=5)[:, :, 4, :],
            ag1_v[:, :, 4, :])

        # lhsT accessors (stationary [128, B] k-tiles, transposed activations)
        def lhs_text(k):
            return acts_all[:, (k // 4) * 5 + (k % 4), :]

        def lhs_audio(k):
            return acts_all[:, k * 5 + 4, :]

        def lhs_video(k):
            return vt_sb[:, k, :]

        def gemm(out_ps, wname, n_kt, lhs_fn, bias_idx):
            # bias as a K=1 matmul row; also opens the accumulation group
            nc.tensor.matmul(out_ps[:], ones_sb[:], bias_sb[:, bias_idx, :],
                             start=True, stop=False)
            w = w_sb[wname]
            nch = w.shape[1]
            cnt = n_kt // nch
            wv = w.rearrange("p c (a n) -> p c a n", n=S)
            for ch in range(nch):
                for a in range(cnt):
                    k = ch * cnt + a
                    nc.tensor.matmul(out_ps[:], lhs_fn(k), wv[:, ch, a, :],
                                     start=False, stop=(k == n_kt - 1))

        # ---- the six GEMMs + GLU/partial-norm per embed ----
        # (video first: its inputs are local, so it runs during the skew
        # window while AG1 is still in flight)
        PLAN = [("video", "wv", "wgv", KD, lhs_video),
                ("text", "wt", "wgt", KD, lhs_text),
                ("audio", "wa", "wga", KA, lhs_audio)]
        EIDX = {e: i for i, e in enumerate(EMBEDS)}
        for e, wn1, wn2, nkt, lf in PLAN:
            ei = EIDX[e]
            h_ps = psum.tile([B, S], f32, name="h_ps", tag="h_ps")
            gemm(h_ps, wn1, nkt, lf, BIAS_IDX[(e, 1)])
            nc.vector.tensor_copy(h16[e][:], h_ps[:])
            g_ps = psum.tile([B, S], f32, name="g_ps", tag="g_ps")
            gemm(g_ps, wn2, nkt, lf, BIAS_IDX[(e, 2)])
            sg16 = work.tile([B, S], fp16, name="sg16", tag="sg16")
            nc.scalar.activation(sg16[:], g_ps[:], ACTF.Sigmoid)
            nc.vector.tensor_mul(y_sb[e][:], h16[e][:], sg16[:])
            ysq = work.tile([B, S], f32, name="ysq", tag="ysq")
            nc.vector.tensor_mul(ysq[:], y_sb[e][:], y_sb[e][:])
            nc.vector.reduce_sum(nsq[:, ei:ei + 1], ysq[:], AX.X)

        # ---- AllReduce norm partials; normalize; write outputs ----
        ar_in = dram.tile([B, 3], f32)
        ar_out = dram.tile([B, 3], f32, addr_space="Shared")
        nc.gpsimd.dma_start(ar_in[:], nsq[:, 0:3])
        nc.gpsimd.collective_compute(
            "AllReduce", ALU.add, replica_groups=RG,
            ins=[ar_in.opt()], outs=[ar_out.opt()])
        nc.sync.dma_start(nsqg[:], ar_out[:])
        nc.scalar.sqrt(nrm[:], nsqg[:])
        nc.vector.tensor_scalar_max(nrm[:], nrm[:], 1e-12)
        nc.vector.reciprocal(rcp[:], nrm[:])
        # final scaling spread over three engines, outputs on three queues
        oq = [nc.sync, nc.scalar, nc.gpsimd]
        yo = {e: work.tile([B, S], f32, name=f"yo_{e}", bufs=1)
              for e in EMBEDS}
        nc.vector.tensor_scalar_mul(yo["text"][:], y_sb["text"][:],
                                    rcp[:, 0:1])
        nc.scalar.mul(yo["video"][:], y_sb["video"][:], rcp[:, 1:2])
        nc.gpsimd.tensor_scalar_mul(yo["audio"][:], y_sb["audio"][:],
                                    rcp[:, 2:3])
        for ei, e in enumerate(EMBEDS):
            oq[ei].dma_start(out_d[e].ap(), yo[e][:])

    nc.compile()
    return nc


def _get_nc():
    if "nc" not in _STATE:
        _STATE["nc"] = _build()
    return _STATE["nc"]


def _prep_inputs(text, video, audio_feats, Wt, bt, Wgt, bgt, Wv, bv, Wgv, bgv,
                 Wa, ba, Wga, bga, nframes, raw_audio_len):
    """Fuse gating weights, shard + transpose + fp16-cast into per-core maps."""
    f16 = np.float16
    text = np.asarray(text, dtype=np.float32)
    video = np.asarray(video, dtype=np.float32)
    audio = np.asarray(audio_feats, dtype=np.float32)

    Wt = np.asarray(Wt, dtype=np.float32)
    Wgt = np.asarray(Wgt, dtype=np.float32)
    Wv = np.asarray(Wv, dtype=np.float32)
    Wgv = np.asarray(Wgv, dtype=np.float32)
    Wa = np.asarray(Wa, dtype=np.float32)
    Wga = np.asarray(Wga, dtype=np.float32)
    bt = np.asarray(bt, dtype=np.float32)
    bgt = np.asarray(bgt, dtype=np.float32)
    bv = np.asarray(bv, dtype=np.float32)
    bgv = np.asarray(bgv, dtype=np.float32)
    ba = np.asarray(ba, dtype=np.float32)
    bga = np.asarray(bga, dtype=np.float32)

    # gating fusion: g = x @ (Wg@W).T + (bg + Wg@b)
    Wgt_f = Wgt @ Wt
    bgt_f = bgt + Wgt @ bt
    Wgv_f = Wgv @ Wv
    bgv_f = bgv + Wgv @ bv
    Wga_f = Wga @ Wa                     # [D, Da]
    bga_f = bga + Wga @ ba

    ratio = int(round(float(np.asarray(raw_audio_len)) / T))
    nf = np.maximum(
        1, (np.asarray(nframes).astype(np.float32) / ratio).astype(np.int32))
    mask = (np.arange(T)[None, :] < nf[:, None]).astype(np.float32)
    mask = mask / nf[:, None].astype(np.float32)          # [B, T] mask/nf
    maskT = np.ascontiguousarray(mask.T).astype(f16)      # [T, B]

    # video.T pre-tiled to [128, KD, B] (partition-contiguous k-tiles)
    vT = np.ascontiguousarray(
        video.T.reshape(KD, 128, B).transpose(1, 0, 2)).astype(f16)

    def wtile(W, sl):
        """W[sl].T [K, S] -> chunked [nch, 128, cnt*S], contiguous/partition."""
        wt = W[sl, :].T
        kk = wt.shape[0]
        nkt = kk // 128
        nch = max(1, nkt // CH)
        cnt = nkt // nch
        return np.ascontiguousarray(
            wt.reshape(nch, cnt, 128, S).transpose(0, 2, 1, 3)
            .reshape(nch, 128, cnt * S)).astype(f16)

    in_maps = []
    for c in range(NCORES):
        sl = slice(c * S, (c + 1) * S)
        sla = slice(c * SA, (c + 1) * SA)
        m = {
            "wt": wtile(Wt, sl),
            "wgt": wtile(Wgt_f, sl),
            "wv": wtile(Wv, sl),
            "wgv": wtile(Wgv_f, sl),
            "wga": wtile(Wga_f, sl),
            "wa": wtile(Wa, sl),
            "textT": np.ascontiguousarray(
                text[:, :, sl].transpose(2, 0, 1)).astype(f16),
            "audioT": np.ascontiguousarray(
                audio[:, sla, :].transpose(2, 0, 1)).astype(f16),
            "vT": vT,
            "maskT": maskT,
            "biases": np.stack([
                b[sl] for b in (bt, bgt_f, bv, bgv_f, ba, bga_f)
            ]).reshape(1, -1).astype(f16),
        }
        in_maps.append(m)
    return in_maps


def kernel(text, video, audio_feats, Wt, bt, Wgt, bgt, Wv, bv, Wgv, bgv,
           Wa, ba, Wga, bga, nframes, raw_audio_len):
    from concourse.bass_utils import run_bass_kernel_spmd

    nc = _get_nc()
    in_maps = _prep_inputs(text, video, audio_feats, Wt, bt, Wgt, bgt,
                           Wv, bv, Wgv, bgv, Wa, ba, Wga, bga,
                           nframes, raw_audio_len)
    res = run_bass_kernel_spmd(nc, in_maps, list(range(NCORES)))
    _STATE["last_results"] = res
    outs = []
    for e in ("text", "video", "audio"):
        outs.append(np.concatenate(
            [res.results[c][f"out_{e}"] for c in range(NCORES)], axis=1))
    return tuple(outs)


# revision 8
# speedup vs baseline: 1.3535x; 1.0736x over previous
"""Trainium2 Bass kernel for the three-GEU (text/video/audio) embedding model.

Strategy (8 NeuronCores, one chip):
  - Tensor-parallel column sharding: core c owns output columns [512c, 512(c+1))
    of every linear; it reads only its 1/8 slice of each weight matrix.
  - Gating fusion (host): g = (x@W.T+b)@Wg.T+bg == x@(Wg@W).T + (bg+Wg@b),
    so the gating GEMM reads the same gathered activations as the value GEMM.
    This removes the mid-kernel h-transpose + AllGather of the baseline and
    shrinks the audio gating weight from [4096,4096] to [4096,1024].
  - Preprocessing (text max-pool over L, audio ragged masked-mean over T) is
    sharded over the feature dim, computed in transposed layout, and an
    AllGather assembles the full [K, B] activations every core needs as the
    matmul stationary operand. This is the FIRST collective: it absorbs the
    inter-core launch skew while the weight stream saturates DMA.
  - Per embed: h = GEMM(x, Wslice), g = GEMM(x, Wfused_slice), y = h*sig(g),
    partial sum(y^2); one tiny AllReduce of the three norm partials, then
    reciprocal-sqrt scaling on-device.
  - fp16 operands into the PE (fp32 PSUM accumulation), fp32 outputs.
  - Video's GEMMs depend only on local inputs, so they run before the
    AllGather completes; PE order: audio matvecs, video h+g, text h+g,
    audio h+g (weights stream in that consumption order).
"""

import numpy as np

B = 64
L = 30
D = 4096
DA = 1024
T = 128
NCORES = 8
S = D // NCORES     # 512: per-core output shard of D
SA = DA // NCORES   # 128: per-core shard of Da
KD = D // 128       # 32 k-tiles over D
KA = DA // 128      # 8 k-tiles over Da
CH = 16             # k-tiles per weight DMA chunk (16 * 128 * 512 * 2B = 2 MiB)

_STATE: dict = {}


def _build():
    from contextlib import ExitStack

    import concourse.bass as bass
    import concourse.tile as tile
    from concourse import bacc, mybir

    fp16 = mybir.dt.float16
    f32 = mybir.dt.float32
    AX = mybir.AxisListType
    ALU = mybir.AluOpType
    ACTF = mybir.ActivationFunctionType

    nc = bacc.Bacc(
        "TRN2",
        target_bir_lowering=False,
        debug=False,
        enable_asserts=False,
        num_devices=NCORES,
    )
    RG = [list(range(NCORES))]

    # --- kernel I/O (per-core shards, staged by the host wrapper) ---
    # Weights arrive pre-tiled [n_chunks, 128, CH, S] so each chunk DMA is
    # fully contiguous per partition (16 KB).
    w_in = {}
    for name, kk in [("wv", D), ("wgv", D), ("wa", DA), ("wga", DA),
                     ("wt", D), ("wgt", D)]:
        nkt = kk // 128
        nch = max(1, nkt // CH)
        w_in[name] = nc.dram_tensor(
            name, [nch, 128, (nkt // nch) * S], fp16, kind="ExternalInput")
    textT = nc.dram_tensor("textT", [S, B, L], fp16, kind="ExternalInput")
    audioT = nc.dram_tensor("audioT", [T, B, SA], fp16, kind="ExternalInput")
    vT_d = nc.dram_tensor("vT", [128, KD, B], fp16, kind="ExternalInput")
    maskT_d = nc.dram_tensor("maskT", [T, B], fp16, kind="ExternalInput")
    biases_d = nc.dram_tensor("biases", [1, 6 * S], fp16, kind="ExternalInput")
    EMBEDS = ("text", "video", "audio")
    out_d = {
        e: nc.dram_tensor(f"out_{e}", [B, S], f32, kind="ExternalOutput")
        for e in EMBEDS
    }

    BIAS_IDX = {("text", 1): 0, ("text", 2): 1, ("video", 1): 2,
                ("video", 2): 3, ("audio", 1): 4, ("audio", 2): 5}

    with ExitStack() as ctx:
        tc = ctx.enter_context(tile.TileContext(nc))

        persist = ctx.enter_context(tc.tile_pool(name="persist", bufs=1))
        work = ctx.enter_context(tc.tile_pool(name="work", bufs=2))
        psum = ctx.enter_context(tc.tile_pool(name="psum", bufs=2, space="PSUM"))
        dram = ctx.enter_context(tc.tile_pool(name="dram", bufs=1, space="DRAM"))

        # ---- persistent SBUF tiles ----
        # All weights live in SBUF for the whole kernel (18 MB of the 26 MB
        # budget) so the PE never waits on a buffer-rotation dependency.
        w_sb = {}
        for name in ("wv", "wgv", "wa", "wga", "wt", "wgt"):
            nch, _, csz = w_in[name].shape
            w_sb[name] = persist.tile([128, nch, csz], fp16, name=f"w_{name}")
        acts_all = persist.tile([128, 40, B], fp16)   # AG result: 8 cores x (4 text + 1 audio) k-tiles
        au_sb = persist.tile([T, B, SA], fp16)        # audio shard, [t, b, c']
        vt_sb = persist.tile([128, KD, B], fp16)      # video.T k-tiles
        msk_sb = persist.tile([T, B], fp16)           # mask/nf, transposed
        bias_sb = persist.tile([1, 6, S], fp16)
        ones_sb = persist.tile([1, B], fp16)
        stg = persist.tile([128, 5, B], fp16)         # AG staging: 4 text + 1 audio k-tile rows
        nsq = persist.tile([B, 4], f32)               # partial sum(y^2) per embed
        nsqg = persist.tile([B, 3], f32)              # AllReduce result
        nrm = persist.tile([B, 3], f32)
        rcp = persist.tile([B, 3], f32)
        h16 = {e: persist.tile([B, S], fp16, name=f"h16_{e}") for e in EMBEDS}
        y_sb = {e: persist.tile([B, S], f32, name=f"y_{e}") for e in EMBEDS}

        # ---- constants ----
        nc.vector.memset(ones_sb[:], 1.0)
        nc.vector.memset(nsq[:], 0.0)

        # ---- input DMAs: audio/mask first on sync, video/bias first on
        # scalar (both HWDGE queues), text on the SWDGE ring ----
        nc.sync.dma_start(au_sb[:], audioT.ap())
        nc.sync.dma_start(msk_sb[:], maskT_d.ap())
        nc.scalar.dma_start(bias_sb[0:1, :, :], biases_d.ap())
        nc.scalar.dma_start(vt_sb[:], vT_d.ap())

        # ---- weight stream: all chunks up-front, in PE consumption order,
        # alternating between the two HWDGE queues ----
        hwdge = [nc.sync, nc.scalar]
        qi = 0
        for name in ("wv", "wgv", "wt", "wgt", "wa", "wga"):
            nch = w_in[name].shape[0]
            for ch in range(nch):
                hwdge[qi % 2].dma_start(
                    w_sb[name][:, ch, :], w_in[name].ap()[ch])
                qi += 1

        # ---- text max-pool over L (sharded over d) -> stg[:, 0:4, :] ----
        t_view = textT.ap().rearrange("(n p) b l -> n p b l", p=128)
        for i in range(4):
            tx = work.tile([128, B, L], fp16, name="tx", bufs=2)
            nc.gpsimd.dma_start(tx[:], t_view[i])
            nc.vector.reduce_max(stg[:, i, :], tx[:], AX.X)

        # ---- audio ragged masked-mean (sharded over Da): 64 PE matvecs ----
        aT_ps = psum.tile([SA, B], f32, bufs=1)
        for b in range(B):
            nc.tensor.matmul(
                aT_ps[:, b:b + 1], au_sb[:, b, :], msk_sb[:, b:b + 1],
                start=True, stop=True)
        nc.vector.tensor_copy(stg[:, 4, :], aT_ps[:])

        # ---- AllGather the preprocessed activations ----
        # Row p of ag1_in carries partition p's 5 staged k-tile rows flat
        # (5*B els), so staging and reload DMAs are contiguous per partition.
        ag1_in = dram.tile([128, 5 * B], fp16)
        ag1_out = dram.tile([128 * NCORES, 5 * B], fp16, addr_space="Shared")
        nc.gpsimd.dma_start(ag1_in[:], stg[:])
        nc.gpsimd.collective_compute(
            "AllGather", ALU.bypass, replica_groups=RG,
            ins=[ag1_in.opt()], outs=[ag1_out.opt()])
        # reload text rows first (text GEMMs run right after the gather;
        # audio's come last)
        ag1_v = ag1_out.rearrange("(r p) (j b) -> p r j b", p=128, b=B)
        nc.gpsimd.dma_start(
            acts_all.rearrange("p (r j) b -> p r j b", j=5)[:, :, 0:4, :],
            ag1_v[:, :, 0:4, :])
        nc.gpsimd.dma_start(
            acts_all.rearrange("p (r j) b -> p r j b", j=5)[:, :, 4, :],
            ag1_v[:, :, 4, :])

        # lhsT accessors (stationary [128, B] k-tiles, transposed activations)
        def lhs_text(k):
            return acts_all[:, (k // 4) * 5 + (k % 4), :]

        def lhs_audio(k):
            return acts_all[:, k * 5 + 4, :]

        def lhs_video(k):
            return vt_sb[:, k, :]

        def gemm(out_ps, wname, n_kt, lhs_fn, bias_idx):
            # bias as a K=1 matmul row; also opens the accumulation group
            nc.tensor.matmul(out_ps[:], ones_sb[:], bias_sb[:, bias_idx, :],
                             start=True, stop=False)
            w = w_sb[wname]
            nch = w.shape[1]
            cnt = n_kt // nch
            wv = w.rearrange("p c (a n) -> p c a n", n=S)
            for ch in range(nch):
                for a in range(cnt):
                    k = ch * cnt + a
                    nc.tensor.matmul(out_ps[:], lhs_fn(k), wv[:, ch, a, :],
                                     start=False, stop=(k == n_kt - 1))

        # ---- the six GEMMs + GLU/partial-norm per embed ----
        # (video first: its inputs are local, so it runs during the skew
        # window while AG1 is still in flight)
        PLAN = [("video", "wv", "wgv", KD, lhs_video),
                ("text", "wt", "wgt", KD, lhs_text),
                ("audio", "wa", "wga", KA, lhs_audio)]
        EIDX = {e: i for i, e in enumerate(EMBEDS)}
        for e, wn1, wn2, nkt, lf in PLAN:
            ei = EIDX[e]
            h_ps = psum.tile([B, S], f32, name="h_ps", tag="h_ps")
            gemm(h_ps, wn1, nkt, lf, BIAS_IDX[(e, 1)])
            nc.vector.tensor_copy(h16[e][:], h_ps[:])
            g_ps = psum.tile([B, S], f32, name="g_ps", tag="g_ps")
            gemm(g_ps, wn2, nkt, lf, BIAS_IDX[(e, 2)])
            sg16 = work.tile([B, S], fp16, name="sg16", tag="sg16")
            nc.scalar.activation(sg16[:], g_ps[:], ACTF.Sigmoid)
            nc.vector.tensor_mul(y_sb[e][:], h16[e][:], sg16[:])
            ysq = work.tile([B, S], f32, name="ysq", tag="ysq")
            nc.vector.tensor_mul(ysq[:], y_sb[e][:], y_sb[e][:])
            nc.vector.reduce_sum(nsq[:, ei:ei + 1], ysq[:], AX.X)

        # ---- AllReduce norm partials; normalize; write outputs ----
        ar_in = dram.tile([B, 3], f32)
        ar_out = dram.tile([B, 3], f32, addr_space="Shared")
        nc.gpsimd.dma_start(ar_in[:], nsq[:, 0:3])
        nc.gpsimd.collective_compute(
            "AllReduce", ALU.add, replica_groups=RG,
            ins=[ar_in.opt()], outs=[ar_out.opt()])
        nc.gpsimd.dma_start(nsqg[:], ar_out[:])
        nc.scalar.sqrt(nrm[:], nsqg[:])
        nc.vector.tensor_scalar_max(nrm[:], nrm[:], 1e-12)
        nc.vector.reciprocal(rcp[:], nrm[:])
        oq = [nc.sync, nc.scalar, nc.gpsimd]
        for ei, e in enumerate(EMBEDS):
            yo = work.tile([B, S], f32, name="yo", tag="yo")
            nc.vector.tensor_scalar_mul(yo[:], y_sb[e][:],
                                        rcp[:, ei:ei + 1])
            oq[ei].dma_start(out_d[e].ap(), yo[:])

    nc.compile()
    return nc


def _get_nc():
    if "nc" not in _STATE:
        _STATE["nc"] = _build()
    return _STATE["nc"]


def _prep_inputs(text, video, audio_feats, Wt, bt, Wgt, bgt, Wv, bv, Wgv, bgv,
                 Wa, ba, Wga, bga, nframes, raw_audio_len):
    """Fuse gating weights, shard + transpose + fp16-cast into per-core maps."""
    f16 = np.float16
    text = np.asarray(text, dtype=np.float32)
    video = np.asarray(video, dtype=np.float32)
    audio = np.asarray(audio_feats, dtype=np.float32)

    Wt = np.asarray(Wt, dtype=np.float32)
    Wgt = np.asarray(Wgt, dtype=np.float32)
    Wv = np.asarray(Wv, dtype=np.float32)
    Wgv = np.asarray(Wgv, dtype=np.float32)
    Wa = np.asarray(Wa, dtype=np.float32)
    Wga = np.asarray(Wga, dtype=np.float32)
    bt = np.asarray(bt, dtype=np.float32)
    bgt = np.asarray(bgt, dtype=np.float32)
    bv = np.asarray(bv, dtype=np.float32)
    bgv = np.asarray(bgv, dtype=np.float32)
    ba = np.asarray(ba, dtype=np.float32)
    bga = np.asarray(bga, dtype=np.float32)

    # gating fusion: g = x @ (Wg@W).T + (bg + Wg@b)
    Wgt_f = Wgt @ Wt
    bgt_f = bgt + Wgt @ bt
    Wgv_f = Wgv @ Wv
    bgv_f = bgv + Wgv @ bv
    Wga_f = Wga @ Wa                     # [D, Da]
    bga_f = bga + Wga @ ba

    ratio = int(round(float(np.asarray(raw_audio_len)) / T))
    nf = np.maximum(
        1, (np.asarray(nframes).astype(np.float32) / ratio).astype(np.int32))
    mask = (np.arange(T)[None, :] < nf[:, None]).astype(np.float32)
    mask = mask / nf[:, None].astype(np.float32)          # [B, T] mask/nf
    maskT = np.ascontiguousarray(mask.T).astype(f16)      # [T, B]

    # video.T pre-tiled to [128, KD, B] (partition-contiguous k-tiles)
    vT = np.ascontiguousarray(
        video.T.reshape(KD, 128, B).transpose(1, 0, 2)).astype(f16)

    def wtile(W, sl):
        """W[sl].T [K, S] -> chunked [nch, 128, cnt*S], contiguous/partition."""
        wt = W[sl, :].T
        kk = wt.shape[0]
        nkt = kk // 128
        nch = max(1, nkt // CH)
        cnt = nkt // nch
        return np.ascontiguousarray(
            wt.reshape(nch, cnt, 128, S).transpose(0, 2, 1, 3)
            .reshape(nch, 128, cnt * S)).astype(f16)

    in_maps = []
    for c in range(NCORES):
        sl = slice(c * S, (c + 1) * S)
        sla = slice(c * SA, (c + 1) * SA)
        m = {
            "wt": wtile(Wt, sl),
            "wgt": wtile(Wgt_f, sl),
            "wv": wtile(Wv, sl),
            "wgv": wtile(Wgv_f, sl),
            "wga": wtile(Wga_f, sl),
            "wa": wtile(Wa, sl),
            "textT": np.ascontiguousarray(
                text[:, :, sl].transpose(2, 0, 1)).astype(f16),
            "audioT": np.ascontiguousarray(
                audio[:, sla, :].transpose(2, 0, 1)).astype(f16),
            "vT": vT,
            "maskT": maskT,
            "biases": np.stack([
                b[sl] for b in (bt, bgt_f, bv, bgv_f, ba, bga_f)
            ]).reshape(1, -1).astype(f16),
        }
        in_maps.append(m)
    return in_maps


def kernel(text, video, audio_feats, Wt, bt, Wgt, bgt, Wv, bv, Wgv, bgv,
           Wa, ba, Wga, bga, nframes, raw_audio_len):
    from concourse.bass_utils import run_bass_kernel_spmd

    nc = _get_nc()
    in_maps = _prep_inputs(text, video, audio_feats, Wt, bt, Wgt, bgt,
                           Wv, bv, Wgv, bgv, Wa, ba, Wga, bga,
                           nframes, raw_audio_len)
    res = run_bass_kernel_spmd(nc, in_maps, list(range(NCORES)))
    _STATE["last_results"] = res
    outs = []
    for e in ("text", "video", "audio"):
        outs.append(np.concatenate(
            [res.results[c][f"out_{e}"] for c in range(NCORES)], axis=1))
    return tuple(outs)
